# revision 1
# baseline (speedup 1.0000x reference)
"""Trainium2 Bass kernel for nn_MultiHeadCrossAttention_47519518163418.

Sharding: 8 cores = (batch b in {0,1}) x (head h in {0..3}); core c: b=c//4, h=c%4.
Each core computes q/k/v for its head's 32 channels (conv output channels are
independent), runs the full 4096x4096 attention for that head (flash-style,
scores computed transposed so no transposes of the score matrix are needed,
softmax without max-subtraction, row sums via an appended ones-column in the
PV matmul), then the cores of a batch AllGather the attention output to form
the full 128-channel mha2d. Green (upsample+conv+ILN+silu) and purple
(upsample+conv+ILN+sigmoid, gated by s) paths are computed per-core for the
core's 32 output channels using a phase-collapsed 2x2-tap decomposition of
"upsample2 + reflect-pad + 3x3 conv" (which reduces to edge-clamp padding on
the original-resolution image). ILN layer statistics are combined with one
tiny AllReduce. Host precomputes positional encodings + paddings and
reassembles the sharded outputs.
"""

import sys

if "/opt/trn_rl_repo" not in sys.path:
    sys.path.insert(0, "/opt/trn_rl_repo")

import numpy as np

NUM_HEADS = 4
EPS = 1e-5
D_HEAD = 32
SCALE = float(D_HEAD) ** -0.5
N_PX = 16384.0          # pixels per channel of the upsampled image
N_TOT = 128 * 16384.0   # elements per batch for layer stats

_CORES = list(range(8))
_REPLICA_GROUPS = [[0, 1, 2, 3], [4, 5, 6, 7]]


# ----------------------------------------------------------------------------
# Host-side helpers
# ----------------------------------------------------------------------------

def pos_encoding_pe(c, L, dtype=np.float32):
    half = c // 2
    pos = np.arange(L, dtype=dtype)
    depths = np.arange(half, dtype=dtype) / half
    rates = 1.0 / (10000.0 ** depths)
    ang = pos[:, None] * rates[None, :]
    pe = np.concatenate([np.sin(ang), np.cos(ang)], axis=-1)  # [L, c]
    return pe.T.astype(dtype)  # [c, L]


def reflect_pad(x):
    return np.pad(x, ((0, 0), (1, 1), (1, 1)), mode="reflect")


def edge_pad(x):
    return np.pad(x, ((0, 0), (1, 1), (1, 1)), mode="edge")


_KSET = {(0, 0): [0], (0, 1): [1, 2], (1, 0): [0, 1], (1, 1): [2]}


def collapse_w2(w):
    """w [co, ci, 3, 3] -> W2 [4 (p=2*pr+pc), 2 (dy), 2 (dx), ci, co]."""
    co, ci = w.shape[0], w.shape[1]
    W2 = np.zeros((4, 2, 2, ci, co), dtype=w.dtype)
    for pr in range(2):
        for pc in range(2):
            p = 2 * pr + pc
            for dy in range(2):
                for dx in range(2):
                    acc = np.zeros((co, ci), dtype=np.float64)
                    for ky in _KSET[(pr, dy)]:
                        for kx in _KSET[(pc, dx)]:
                            acc = acc + w[:, :, ky, kx].astype(np.float64)
                    W2[p, dy, dx] = acc.T.astype(w.dtype)
    return W2


def arrange_to_strips(x2d):
    """x [32, 128, 128] -> arranged [128, 4096] phase-major: partition
    32*(2*pr+pc)+c, free r*64+cc for upsampled pixel (2r+pr, 2cc+pc)."""
    t = x2d.reshape(32, 64, 2, 64, 2)          # c, r, pr, cc, pc
    t = t.transpose(2, 4, 0, 1, 3)              # pr, pc, c, r, cc
    return np.ascontiguousarray(t.reshape(128, 4096))


def unarrange_from_strips(arr):
    t = arr.reshape(2, 2, 32, 64, 64)           # pr, pc, c, r, cc
    t = t.transpose(2, 3, 0, 4, 1)              # c, r, pr, cc, pc
    return np.ascontiguousarray(t.reshape(32, 128, 128))


_PE_Y = None
_PE_S = None


_BATCH_CACHE = {}


def _batch_shared(inputs, b):
    """Padded/PE-added tensors shared by the 4 cores of a batch."""
    key = (id(inputs), b)
    if key in _BATCH_CACHE:
        return _BATCH_CACHE[key]
    y = np.asarray(inputs["y"], dtype=np.float32)[b]
    s = np.asarray(inputs["s"], dtype=np.float32)[b]
    ypepad = np.ascontiguousarray(
        reflect_pad((y + _PE_Y).astype(np.float32)).reshape(2, 128, 66, 66))
    yreppad = np.ascontiguousarray(edge_pad(y).reshape(2, 128, 66, 66))
    spepad = np.ascontiguousarray(reflect_pad((s + _PE_S).astype(np.float32)))
    _BATCH_CACHE.clear()
    _BATCH_CACHE[key] = (ypepad, yreppad, spepad)
    return _BATCH_CACHE[key]


def prepare_core_inputs(inputs, core):
    global _PE_Y, _PE_S
    if _PE_Y is None:
        _PE_Y = pos_encoding_pe(256, 64 * 64).reshape(256, 64, 64)
        _PE_S = pos_encoding_pe(128, 128 * 128).reshape(128, 128, 128)
    b, h = core // 4, core % 4
    ch = slice(32 * h, 32 * h + 32)
    s = np.asarray(inputs["s"], dtype=np.float32)[b]

    ypepad, yreppad, spepad = _batch_shared(inputs, b)
    sgate = arrange_to_strips(np.ascontiguousarray(s[ch]))

    w_blue_y = np.asarray(inputs["w_blue_y"], dtype=np.float32)[ch]
    w_blue_s = np.asarray(inputs["w_blue_s"], dtype=np.float32)[ch]
    w_green = np.asarray(inputs["w_green"], dtype=np.float32)[ch]
    w_purple = np.asarray(inputs["w_purple"], dtype=np.float32)[ch]

    wq = np.zeros((18, 128, 128), dtype=np.float32)
    for t in range(9):
        ky, kx = t // 3, t % 3
        for kt in range(2):
            blk = w_blue_y[:, 128 * kt : 128 * kt + 128, ky, kx].T
            wq[t * 2 + kt] = np.tile(blk, (1, 4))
    wv = np.zeros((9, 128, 32), dtype=np.float32)
    for t in range(9):
        ky, kx = t // 3, t % 3
        wv[t] = w_blue_s[:, :, ky, kx].T

    def make_w9(w):
        # W9[ey, ex][ci, 32*p+c] = W2[p, ey-pr, ex-pc][ci, c] (0 if invalid):
        # all four phases computed from one 9-tap pass over the edge-padded
        # original-resolution image, phase-major on output partitions.
        W2 = collapse_w2(w)                      # [4, 2, 2, ci, co32]
        ci = W2.shape[3]
        W9 = np.zeros((3, 3, ci, 128), dtype=np.float32)
        for p in range(4):
            pr, pc = p // 2, p % 2
            for dy in range(2):
                for dx in range(2):
                    W9[pr + dy, pc + dx, :, 32 * p : 32 * p + 32] = W2[p, dy, dx]
        return W9

    W9g = make_w9(w_green)                       # [3, 3, 256, 128]
    wg = W9g.reshape(3, 3, 2, 128, 128).transpose(0, 1, 2, 3, 4).reshape(18, 128, 128).copy()
    W9p = make_w9(w_purple)                      # [3, 3, 128, 128]
    wp = W9p.reshape(9, 128, 128).copy()

    affg = np.stack(
        [np.asarray(inputs["rho_g"], dtype=np.float32).reshape(128)[ch],
         np.asarray(inputs["gamma_g"], dtype=np.float32).reshape(128)[ch],
         np.asarray(inputs["beta_g"], dtype=np.float32).reshape(128)[ch]],
        axis=1)
    affp = np.stack(
        [np.asarray(inputs["rho_p"], dtype=np.float32).reshape(128)[ch],
         np.asarray(inputs["gamma_p"], dtype=np.float32).reshape(128)[ch],
         np.asarray(inputs["beta_p"], dtype=np.float32).reshape(128)[ch]],
        axis=1)

    sel = np.zeros((128, 32), dtype=np.float32)
    for p in range(128):
        sel[p, p % 32] = 1.0

    return {
        "ypepad": ypepad,
        "yreppad": yreppad,
        "spepad": spepad,
        "sgate": sgate,
        "wq": wq,
        "wv": wv,
        "wg": wg,
        "wp": wp,
        "affg": np.ascontiguousarray(affg),
        "affp": np.ascontiguousarray(affp),
        "sel": sel,
        "onesr": np.ones((128, 1), dtype=np.float32),
    }


def assemble_output(per_core_z, per_core_upy):
    out = np.zeros((2, 256, 128, 128), dtype=np.float32)
    for core in range(8):
        b, h = core // 4, core % 4
        out[b, 32 * h : 32 * h + 32] = unarrange_from_strips(per_core_z[core])
        out[b, 128 + 32 * h : 128 + 32 * h + 32] = unarrange_from_strips(per_core_upy[core])
    return out


# ----------------------------------------------------------------------------
# Bass kernel
# ----------------------------------------------------------------------------

def build_bass(loop_n=None, no_cc=False):
    import concourse.bass as bass
    import concourse.tile as tile
    from concourse import bacc, mybir

    f32 = mybir.dt.float32
    f32r = mybir.dt.float32r
    AF = mybir.ActivationFunctionType
    ALU = mybir.AluOpType

    def r32(ap):
        return ap.bitcast(f32r)

    nc = bacc.Bacc(num_devices=8)

    # ---- I/O ----
    ypepad_d = nc.declare_dram_parameter("ypepad", [2, 128, 66, 66], f32r, isOutput=False)
    yreppad_d = nc.declare_dram_parameter("yreppad", [2, 128, 66, 66], f32r, isOutput=False)
    spepad_d = nc.declare_dram_parameter("spepad", [128, 130, 130], f32r, isOutput=False)
    sgate_d = nc.declare_dram_parameter("sgate", [128, 4096], f32, isOutput=False)
    wq_d = nc.declare_dram_parameter("wq", [18, 128, 128], f32r, isOutput=False)
    wv_d = nc.declare_dram_parameter("wv", [9, 128, 32], f32r, isOutput=False)
    wg_d = nc.declare_dram_parameter("wg", [18, 128, 128], f32r, isOutput=False)
    wp_d = nc.declare_dram_parameter("wp", [9, 128, 128], f32r, isOutput=False)
    affg_d = nc.declare_dram_parameter("affg", [32, 3], f32, isOutput=False)
    affp_d = nc.declare_dram_parameter("affp", [32, 3], f32, isOutput=False)
    sel_d = nc.declare_dram_parameter("sel", [128, 32], f32, isOutput=False)
    onesr_d = nc.declare_dram_parameter("onesr", [128, 1], f32r, isOutput=False)
    zout_d = nc.declare_dram_parameter("zout", [128, 4096], f32, isOutput=True)
    upyout_d = nc.declare_dram_parameter("upyout", [128, 4096], f32, isOutput=True)

    # ---- internal DRAM (collective bounce buffers etc.) ----
    cc1_in = nc.dram_tensor("cc1_in", [32, 4096], f32r)
    cc1_out = nc.dram_tensor("cc1_out", [128, 4096], f32r)
    cc2_in = nc.dram_tensor("cc2_in", [1, 4], f32)
    cc2_out = nc.dram_tensor("cc2_out", [1, 4], f32)
    rsumb = nc.dram_tensor("rsumb", [4096], f32)
    rsumb2 = nc.dram_tensor("rsumb2", [4096], f32)

    import contextlib

    with tile.TileContext(nc) as tc, contextlib.ExitStack() as ctx:
        pers = ctx.enter_context(tc.tile_pool(name="pers", bufs=1))
        small = ctx.enter_context(tc.tile_pool(name="small", bufs=1))

        # ---------------- weights + constants ----------------
        wq_sb = pers.tile([128, 18, 128], f32r, tag="wq")
        nc.sync.dma_start(out=wq_sb, in_=wq_d[:, :, :].rearrange("t p m -> p t m"))
        wv_sb = pers.tile([128, 9, 32], f32r, tag="wv")
        nc.sync.dma_start(out=wv_sb, in_=wv_d[:, :, :].rearrange("t p m -> p t m"))
        wg_sb = pers.tile([128, 18, 128], f32r, tag="wg")
        nc.sync.dma_start(out=wg_sb, in_=wg_d[:, :, :].rearrange("t p m -> p t m"))
        wp_sb = pers.tile([128, 9, 128], f32r, tag="wp")
        nc.sync.dma_start(out=wp_sb, in_=wp_d[:, :, :].rearrange("t p m -> p t m"))
        sel_sb = pers.tile([128, 32], f32, tag="sel")
        nc.sync.dma_start(out=sel_sb, in_=sel_d[:, :])
        ones_sb = pers.tile([128, 1], f32, tag="ones")
        nc.vector.memset(ones_sb, 1.0)
        affg_sb = small.tile([32, 3], f32, tag="affg")
        nc.sync.dma_start(out=affg_sb, in_=affg_d[:, :])
        affp_sb = small.tile([32, 3], f32, tag="affp")
        nc.sync.dma_start(out=affp_sb, in_=affp_d[:, :])

        def rsqrt_col(x, p, tag, eps=EPS):
            """[p, 1] tile -> rsqrt(x + eps), via reciprocal + Sqrt ACT +
            one Newton step (y*(1.5 - 0.5*xe*y^2)) to clean up sqrt ULPs."""
            xe = small.tile([p, 1], f32, tag=tag + "xe", name=tag + "xe")
            nc.vector.tensor_scalar_add(xe, x, eps)
            r = small.tile([p, 1], f32, tag=tag + "r", name=tag + "r")
            nc.vector.reciprocal(out=r, in_=xe)
            y = small.tile([p, 1], f32, tag=tag + "y", name=tag + "y")
            nc.scalar.activation(out=y, in_=r, func=AF.Sqrt)
            t = small.tile([p, 1], f32, tag=tag + "nt", name=tag + "nt")
            nc.vector.tensor_mul(t, y, y)
            nc.vector.tensor_mul(t, t, xe)
            nc.vector.tensor_scalar(out=t, in0=t, scalar1=-0.5, scalar2=1.5,
                                    op0=ALU.mult, op1=ALU.add)
            nc.vector.tensor_mul(y, y, t)
            return y

        def emit_body():
            attn = ctx.enter_context(tc.tile_pool(name="attn", bufs=1))
            # =========== v conv (blue_s, stride 2, M=32) ===========
            vT_sb = attn.tile([128, 33 * 32], f32r, tag="vT")
            vT_ones_view = vT_sb.rearrange("p (jb c) -> p jb c", c=33)[:, :, 32:33]
            nc.sync.dma_start(
                out=vT_ones_view,
                in_=bass.AP(tensor=onesr_d, offset=0, ap=[[1, 128], [0, 32], [0, 1]]))
            vstats = small.tile([32, 8, 6], f32, tag="vstats")
            with tc.tile_pool(name="vsec", bufs=1) as vsec, \
                 tc.tile_pool(name="vtmp", bufs=2) as vtmp_pool, \
                 tc.tile_pool(name="cps2", bufs=3, space="PSUM") as cps2:
                spe = vsec.tile([128, 130, 130], f32r, tag="spe")
                for rb in range(5):
                    r0, r1 = 26 * rb, 26 * rb + 26
                    nc.sync.dma_start(out=spe[:, r0:r1, :], in_=spepad_d[:, r0:r1, :])
                vraw = vtmp_pool.tile([32, 4096], f32, tag="vtmp", name="vraw")
                for chunk in range(8):
                    vps = cps2.tile([128, 512], f32, tag="vps")
                    r0 = 8 * chunk
                    for t in range(9):
                        ky, kx = t // 3, t % 3
                        nc.tensor.matmul(
                            vps[0:32, :],
                            wv_sb[:, t, :],
                            spe[:, 2 * r0 + ky : 2 * r0 + ky + 16 : 2, kx : kx + 128 : 2],
                            start=(t == 0), stop=(t == 8),
                        )
                    nc.vector.tensor_copy(vraw[:, 512 * chunk : 512 * chunk + 512], vps[0:32, :])
                    nc.vector.bn_stats(out=vstats[:, chunk, :], in_=vraw[:, 512 * chunk : 512 * chunk + 512])

                vmv = small.tile([32, 2], f32, tag="vmv")
                nc.vector.bn_aggr(out=vmv, in_=vstats)
                vinv = rsqrt_col(vmv[:, 1:2], 32, "vinv")
                vbias = small.tile([32, 1], f32, tag="vbias")
                nc.vector.tensor_scalar(out=vbias, in0=vmv[:, 0:1], scalar1=vinv, scalar2=-1.0,
                                        op0=ALU.mult, op1=ALU.mult)
                v2d = vtmp_pool.tile([32, 4096], f32, tag="vtmp", name="v2d")
                nc.scalar.activation(out=v2d, in_=vraw, func=AF.Silu, bias=vbias, scale=vinv)

                # vT with ones column: vT_sb[32w+i, 33*jb+c] = v2d[c, 128*jb+32w+i]
                vt32 = vtmp_pool.tile([32, 4096], f32, tag="vtmp", name="vt32")
                nc.vector.transpose(out=vt32, in_=v2d)
                vt32_v = vt32.rearrange("p (m c) -> p m c", c=32)   # m = 4*jb + w
                vT_v = vT_sb.rearrange("p (jb c) -> p jb c", c=33)
                for w in range(4):
                    nc.gpsimd.dma_start(
                        out=vT_v[32 * w : 32 * w + 32, :, 0:32],
                        in_=vt32_v[:, w::4, :])

            # =========== q conv (blue_y, M=128 replicated) ===========
            qstats = small.tile([128, 8, 6], f32, tag="qstats")
            with tc.tile_pool(name="qsec", bufs=1) as qsec, \
                 tc.tile_pool(name="cps1", bufs=3, space="PSUM") as cps1:
                ype = [qsec.tile([128, 66, 66], f32r, tag=f"ype{kt}", name=f"ype{kt}") for kt in range(2)]
                for kt in range(2):
                    for rb in range(3):
                        r0, r1 = 22 * rb, 22 * rb + 22
                        nc.sync.dma_start(out=ype[kt][:, r0:r1, :], in_=ypepad_d[kt][:, r0:r1, :])
                qraw = qsec.tile([128, 4096], f32, tag="qraw")
                for chunk in range(8):
                    qps = cps1.tile([128, 512], f32, tag="qps")
                    r0 = 8 * chunk
                    idx = 0
                    for t in range(9):
                        ky, kx = t // 3, t % 3
                        for kt in range(2):
                            nc.tensor.matmul(
                                qps[:, :],
                                wq_sb[:, t * 2 + kt, :],
                                ype[kt][:, r0 + ky : r0 + ky + 8, kx : kx + 64],
                                start=(idx == 0), stop=(idx == 17),
                            )
                            idx += 1
                    nc.vector.tensor_copy(qraw[:, 512 * chunk : 512 * chunk + 512], qps[:, :])
                    nc.vector.bn_stats(out=qstats[:, chunk, :], in_=qraw[:, 512 * chunk : 512 * chunk + 512])

                qmv = small.tile([128, 2], f32, tag="qmv")
                nc.vector.bn_aggr(out=qmv, in_=qstats)
                qinv = rsqrt_col(qmv[:, 1:2], 128, "qinv")
                qbias = small.tile([128, 1], f32, tag="qbias")
                nc.vector.tensor_scalar(out=qbias, in0=qmv[:, 0:1], scalar1=qinv, scalar2=-1.0,
                                        op0=ALU.mult, op1=ALU.mult)
                qrep = attn.tile([128, 4096], f32r, tag="qrep")
                nc.scalar.activation(out=qrep, in_=qraw, func=AF.Silu, bias=qbias, scale=qinv)

            # =========== attention + interleaved green conv ===========
            greenraw = ctx.enter_context(tc.tile_pool(name="gpool", bufs=1)).tile(
                [128, 4096], f32, tag="greenraw", name="greenraw")
            gstats = small.tile([128, 8, 6], f32, tag="gstats")
            mharaw = attn.tile([33, 4096], f32, tag="mharaw")

            with tc.tile_pool(name="yrep", bufs=1) as yrep_pool, \
                 tc.tile_pool(name="aexpp", bufs=4) as aexp_pool, \
                 tc.tile_pool(name="gps", bufs=2, space="PSUM") as gps_pool, \
                 tc.tile_pool(name="qkps", bufs=2, space="PSUM") as qkps, \
                 tc.tile_pool(name="pvps", bufs=2, space="PSUM") as pvps:
                yrep = [yrep_pool.tile([128, 66, 66], f32r, tag=f"yrep{kt}", name=f"yrep{kt}") for kt in range(2)]
                for kt in range(2):
                    for rb in range(3):
                        r0, r1 = 22 * rb, 22 * rb + 22
                        nc.sync.dma_start(out=yrep[kt][:, r0:r1, :], in_=yreppad_d[kt][:, r0:r1, :])

                green_tiles = {}

                def green_piece(piece):
                    # piece = (chunk, sub) with sub in 0..5 -> 3 MMs each
                    chunk, sub = piece // 6, piece % 6
                    r0 = 8 * chunk
                    if sub == 0:
                        green_tiles[chunk] = gps_pool.tile(
                            [128, 512], f32, tag="gpsum", name=f"g{chunk}")
                    gtile = green_tiles[chunk]
                    for k in range(3):
                        idx = sub * 3 + k
                        tap, kt = idx // 2, idx % 2
                        ey, ex = tap // 3, tap % 3
                        nc.tensor.matmul(
                            gtile[:, :],
                            wg_sb[:, tap * 2 + kt, :],
                            yrep[kt][:, r0 + ey : r0 + ey + 8, ex : ex + 64],
                            start=(idx == 0), stop=(idx == 17),
                        )
                    if sub == 5:
                        col = 512 * chunk
                        nc.vector.tensor_copy(greenraw[:, col : col + 512], gtile[:, :])
                        nc.vector.bn_stats(out=gstats[:, chunk, :], in_=greenraw[:, col : col + 512])
                        del green_tiles[chunk]

                vT_v = vT_sb.rearrange("p (jb c) -> p jb c", c=33)
                gu_next = 0
                it = 0
                for I in range(8):
                    pvt = pvps.tile([128, 512], f32, tag="pvt", name=f"pvt{I}")
                    for g in range(16):
                        qk = qkps.tile([128, 1024], f32, tag="qk", name=f"qk{I}_{g}")
                        for t in range(2):
                            jb = 2 * g + t
                            nc.tensor.matmul(
                                qk[:, 512 * t : 512 * t + 512],
                                qrep[0:32, 128 * jb : 128 * jb + 128],
                                qrep[0:32, 512 * I : 512 * I + 512],
                                start=True, stop=True,
                            )
                        aexp = aexp_pool.tile([128, 1024], f32r, tag="aexp", name=f"ae{I}_{g}")
                        nc.scalar.activation(out=aexp, in_=qk, func=AF.Exp, scale=SCALE)
                        for t in range(2):
                            jb = 2 * g + t
                            nc.tensor.matmul(
                                pvt[0:33, :],
                                vT_v[:, jb, :],
                                aexp[:, 512 * t : 512 * t + 512],
                                start=(g == 0 and t == 0), stop=(g == 15 and t == 1),
                                skip_group_check=True,
                            )
                        it += 1
                        if it % 2 == 0 and gu_next < 48:
                            green_piece(gu_next)
                            gu_next += 1
                    nc.vector.tensor_copy(mharaw[:, 512 * I : 512 * I + 512], pvt[0:33, :])

            gmv = small.tile([128, 2], f32, tag="gmv")
            nc.vector.bn_aggr(out=gmv, in_=gstats)

            # =========== softmax denominators + divide ===========
            with tc.tile_pool(name="divp", bufs=1) as divp:
                nc.sync.dma_start(out=bass.AP(tensor=rsumb, offset=0, ap=[[1, 4096]]),
                                  in_=mharaw[32:33, :])
                rsq = small.tile([128, 32], f32, tag="rsq")
                nc.sync.dma_start(out=rsq, in_=bass.AP(tensor=rsumb, offset=0, ap=[[32, 128], [1, 32]]))
                nc.vector.reciprocal(out=rsq, in_=rsq)
                nc.sync.dma_start(out=bass.AP(tensor=rsumb2, offset=0, ap=[[32, 128], [1, 32]]), in_=rsq)
                rs32 = divp.tile([32, 4096], f32, tag="rs32")
                nc.sync.dma_start(out=rs32,
                                  in_=bass.AP(tensor=rsumb2, offset=0, ap=[[0, 32], [1, 4096]]))
                mha2db = divp.tile([32, 4096], f32r, tag="mha2db")
                nc.vector.tensor_mul(mha2db, mharaw[0:32, :], rs32)

                # AllGather mha across the 4 cores of this batch
                nc.sync.dma_start(out=cc1_in[:, :], in_=mha2db)
                if no_cc:
                    for g in range(4):
                        nc.sync.dma_start(out=cc1_out[32 * g : 32 * g + 32, :], in_=cc1_in[:, :])
                else:
                    nc.gpsimd.collective_compute(
                        "AllGather", mybir.AluOpType.bypass,
                        replica_groups=_REPLICA_GROUPS,
                        ins=[cc1_in[:, :]],
                        outs=[cc1_out[:, :]],
                    )

            # =========== purple conv ===========
            with tc.tile_pool(name="tailp", bufs=1) as tailp, \
                 tc.tile_pool(name="gps2", bufs=2, space="PSUM") as gps2, \
                 tc.tile_pool(name="tailps", bufs=2, space="PSUM") as tailps:
                mhapad = tailp.tile([128, 66, 66], f32r, tag="mhapad")
                cc1_v = cc1_out.rearrange("p (r c) -> p r c", c=64)
                nc.sync.dma_start(out=mhapad[:, 1:65, 1:65], in_=cc1_v)
                nc.sync.dma_start(out=mhapad[:, 0:1, 1:65], in_=cc1_v[:, 0:1, :])
                nc.sync.dma_start(out=mhapad[:, 65:66, 1:65], in_=cc1_v[:, 63:64, :])
                nc.sync.dma_start(out=mhapad[:, 0:66, 0:1], in_=mhapad[:, 0:66, 1:2])
                nc.sync.dma_start(out=mhapad[:, 0:66, 65:66], in_=mhapad[:, 0:66, 64:65])

                purpleraw = tailp.tile([128, 4096], f32, tag="purpleraw")
                pstats = small.tile([128, 8, 6], f32, tag="pstats")
                for chunk in range(8):
                    ptile = gps2.tile([128, 512], f32, tag="gpsum2", name=f"pt{chunk}")
                    r0 = 8 * chunk
                    for tap in range(9):
                        ey, ex = tap // 3, tap % 3
                        nc.tensor.matmul(
                            ptile[:, :],
                            wp_sb[:, tap, :],
                            mhapad[:, r0 + ey : r0 + ey + 8, ex : ex + 64],
                            start=(tap == 0), stop=(tap == 8),
                        )
                    col = 512 * chunk
                    nc.vector.tensor_copy(purpleraw[:, col : col + 512], ptile[:, :])
                    nc.vector.bn_stats(out=pstats[:, chunk, :], in_=purpleraw[:, col : col + 512])
                pmv = small.tile([128, 2], f32, tag="pmv")
                nc.vector.bn_aggr(out=pmv, in_=pstats)

                # ---- stats -> sums, channel combine, allreduce ----
                def part_sums(mv, tag):
                    s2 = small.tile([128, 2], f32, tag=tag, name=tag)
                    nc.vector.tensor_scalar_mul(s2[:, 0:1], mv[:, 0:1], 4096.0)
                    t = small.tile([128, 1], f32, tag=tag + "t", name=tag + "t")
                    nc.vector.tensor_mul(t, mv[:, 0:1], mv[:, 0:1])
                    nc.vector.tensor_add(t, t, mv[:, 1:2])
                    nc.vector.tensor_scalar_mul(s2[:, 1:2], t, 4096.0)
                    return s2

                gsums2 = part_sums(gmv, "gsums2")
                psums2 = part_sums(pmv, "psums2")

                chps = tailps.tile([128, 512], f32, tag="tps", name="chps")
                nc.tensor.matmul(chps[0:32, 0:2], sel_sb, gsums2, start=True, stop=True)
                gch = small.tile([32, 2], f32, tag="gch")
                nc.vector.tensor_copy(gch, chps[0:32, 0:2])
                chps2 = tailps.tile([128, 512], f32, tag="tps", name="chps2")
                nc.tensor.matmul(chps2[0:32, 0:2], sel_sb, psums2, start=True, stop=True)
                pch = small.tile([32, 2], f32, tag="pch")
                nc.vector.tensor_copy(pch, chps2[0:32, 0:2])

                lps = tailps.tile([128, 512], f32, tag="tps", name="lps")
                nc.tensor.matmul(lps[0:1, 0:2], ones_sb, gsums2, start=True, stop=True)
                nc.tensor.matmul(lps[0:1, 2:4], ones_sb, psums2, start=True, stop=True)
                lsb = small.tile([1, 4], f32, tag="lsb")
                nc.vector.tensor_copy(lsb, lps[0:1, 0:4])
                nc.sync.dma_start(out=cc2_in[:, :], in_=lsb)
                if no_cc:
                    nc.sync.dma_start(out=cc2_out[:, :], in_=cc2_in[:, :])
                else:
                    nc.gpsimd.collective_compute(
                        "AllReduce", mybir.AluOpType.add,
                        replica_groups=_REPLICA_GROUPS,
                        ins=[cc2_in[:, :]],
                        outs=[cc2_out[:, :]],
                    )
                lng = small.tile([32, 4], f32, tag="lng")
                nc.sync.dma_start(out=lng, in_=bass.AP(tensor=cc2_out, offset=0, ap=[[0, 32], [1, 4]]))

                # ---- ILN affines ----
                def iln_affine(ch_sums, S_col, aff_sb, tag):
                    n, n1 = N_PX, N_PX - 1.0
                    nt, nt1 = N_TOT, N_TOT - 1.0
                    in_m = small.tile([32, 1], f32, tag=tag + "im", name=tag + "im")
                    nc.vector.tensor_scalar_mul(in_m, ch_sums[:, 0:1], 1.0 / n)
                    t1 = small.tile([32, 1], f32, tag=tag + "t1", name=tag + "t1")
                    nc.vector.tensor_mul(t1, ch_sums[:, 0:1], ch_sums[:, 0:1])
                    nc.vector.tensor_scalar_mul(t1, t1, 1.0 / n)
                    nc.vector.tensor_sub(t1, ch_sums[:, 1:2], t1)
                    in_v = small.tile([32, 1], f32, tag=tag + "iv", name=tag + "iv")
                    nc.vector.tensor_scalar_mul(in_v, t1, 1.0 / n1)
                    inv_in = rsqrt_col(in_v, 32, tag + "ii")

                    ln_m = small.tile([32, 1], f32, tag=tag + "lm", name=tag + "lm")
                    nc.vector.tensor_scalar_mul(ln_m, S_col[:, 0:1], 1.0 / nt)
                    l1 = small.tile([32, 1], f32, tag=tag + "l1", name=tag + "l1")
                    nc.vector.tensor_mul(l1, S_col[:, 0:1], S_col[:, 0:1])
                    nc.vector.tensor_scalar_mul(l1, l1, 1.0 / nt)
                    nc.vector.tensor_sub(l1, S_col[:, 1:2], l1)
                    ln_v = small.tile([32, 1], f32, tag=tag + "lv", name=tag + "lv")
                    nc.vector.tensor_scalar_mul(ln_v, l1, 1.0 / nt1)
                    inv_ln = rsqrt_col(ln_v, 32, tag + "il")

                    rho = aff_sb[:, 0:1]
                    t3 = small.tile([32, 1], f32, tag=tag + "t3", name=tag + "t3")
                    nc.vector.tensor_mul(t3, rho, inv_in)
                    t6 = small.tile([32, 1], f32, tag=tag + "t6", name=tag + "t6")
                    nc.vector.tensor_mul(t6, rho, inv_ln)
                    nc.vector.tensor_sub(t6, inv_ln, t6)
                    A = small.tile([32, 1], f32, tag=tag + "A", name=tag + "A")
                    nc.vector.tensor_add(A, t3, t6)
                    u1 = small.tile([32, 1], f32, tag=tag + "u1", name=tag + "u1")
                    nc.vector.tensor_mul(u1, in_m, t3)
                    u2 = small.tile([32, 1], f32, tag=tag + "u2", name=tag + "u2")
                    nc.vector.tensor_mul(u2, ln_m, t6)
                    nc.vector.tensor_add(u1, u1, u2)
                    B = small.tile([32, 1], f32, tag=tag + "B", name=tag + "B")
                    nc.vector.tensor_scalar_mul(B, u1, -1.0)
                    sb = small.tile([32, 2], f32, tag=tag + "sb", name=tag + "sb")
                    nc.vector.tensor_mul(sb[:, 0:1], A, aff_sb[:, 1:2])
                    nc.vector.tensor_mul(sb[:, 1:2], B, aff_sb[:, 1:2])
                    nc.vector.tensor_add(sb[:, 1:2], sb[:, 1:2], aff_sb[:, 2:3])
                    return sb

                gsb = iln_affine(gch, lng[:, 0:2], affg_sb, "ga")
                psb = iln_affine(pch, lng[:, 2:4], affp_sb, "pa")

                gsb128 = small.tile([128, 2], f32, tag="gsb128")
                psb128 = small.tile([128, 2], f32, tag="psb128")
                nc.sync.dma_start(out=gsb128[0:32, :], in_=gsb)
                nc.sync.dma_start(out=psb128[0:32, :], in_=psb)
                for o in (32, 64, 96):
                    nc.sync.dma_start(out=gsb128[o : o + 32, :], in_=gsb128[0:32, :])
                    nc.sync.dma_start(out=psb128[o : o + 32, :], in_=psb128[0:32, :])

                # ---- finalize outputs ----
                sgate_sb = tailp.tile([128, 4096], f32, tag="sgate")
                nc.sync.dma_start(out=sgate_sb, in_=sgate_d[:, :])

                upy_sb = tailp.tile([128, 4096], f32, tag="upy")
                nc.scalar.activation(out=upy_sb, in_=greenraw, func=AF.Silu,
                                     bias=gsb128[:, 1:2], scale=gsb128[:, 0:1])
                nc.sync.dma_start(out=upyout_d[:, :], in_=upy_sb)

                zpre = tailp.tile([128, 4096], f32, tag="zpre")
                nc.scalar.activation(out=zpre, in_=purpleraw, func=AF.Sigmoid,
                                     bias=psb128[:, 1:2], scale=psb128[:, 0:1])
                nc.vector.tensor_mul(zpre, zpre, sgate_sb)
                nc.sync.dma_start(out=zout_d[:, :], in_=zpre)

        if loop_n is None:
            emit_body()
        else:
            with tc.For_i(0, loop_n, 1):
                emit_body()

    nc.compile()
    return nc


_NC_CACHE = None
RUN_KWARGS = {}      # test harness may set e.g. {"trace": True}
LAST_RESULTS = None  # BassKernelResults of the most recent run


def kernel(**inputs) -> np.ndarray:
    global _NC_CACHE, LAST_RESULTS
    from concourse.bass_utils import run_bass_kernel_spmd

    if _NC_CACHE is None:
        _NC_CACHE = build_bass()
    nc = _NC_CACHE

    in_maps = []
    for core in _CORES:
        ci = prepare_core_inputs(inputs, core)
        in_maps.append(ci)

    res = run_bass_kernel_spmd(nc, in_maps, _CORES, **RUN_KWARGS)
    LAST_RESULTS = res
    zs = [res.results[c]["zout"] for c in _CORES]
    upys = [res.results[c]["upyout"] for c in _CORES]
    return assemble_output(zs, upys)


if __name__ == "__main__":
    nc = build_bass()
    print("bass build OK")



# revision 22
# speedup vs baseline: 1.0935x; 1.0935x over previous
"""Trainium2 Bass kernel for nn_MultiHeadCrossAttention_47519518163418.

Sharding: 8 cores = (batch b in {0,1}) x (head h in {0..3}); core c: b=c//4, h=c%4.

v2 design (vs v1 baseline):
- All conv inputs/weights in bf16 (halves input DMA; conv error ~0.25%).
- Attention in fp8e4m3 with DoubleRow matmuls: QK contracts d-pairs packed as
  [16,2,*]; PV contracts j-pairs via the (even jb, odd jb) PSUM layout the two
  QK matmuls already produce. exp runs with bias = EXP_C - M*SCALE where
  M = max_i |q_i|^2 (computed on device; Cauchy-Schwarz makes M*SCALE a tight
  upper bound of all scores), so every attention weight lands in fp8's normal
  range; the ones-column in the PV stationary gives row sums from the same
  quantized weights, so fp8 rounding cancels consistently in the division.
- The mha AllGather sends mean-subtracted deviations in bf16 (mha is a large
  per-channel mean + tiny fluctuations; instance norm extracts the
  fluctuations, so quantizing raw values would destroy the signal). The
  per-channel offsets ride in the payload as bitcast f32 and are restored
  after the purple conv as a per-channel constant K = Wsum @ c (edge-padded
  conv of a constant image is constant).
- The gather is split: I-blocks 0..5 gather during the tail of the attention
  loop; blocks 6..7 (+ metadata) gather at loop end, overlapping the first
  purple conv chunks (mhapad is split into two row-bands so chunks 0..3 only
  depend on gather #1).
- Green conv (bf16) is interleaved into the attention loop to fill the PE
  while ACT does exp; green ILN layer sums ride gather #2.
"""

import sys

if "/opt/trn_rl_repo" not in sys.path:
    sys.path.insert(0, "/opt/trn_rl_repo")

import numpy as np
import ml_dtypes

BF16 = ml_dtypes.bfloat16

NUM_HEADS = 4
EPS = 1e-5
D_HEAD = 32
SCALE = float(D_HEAD) ** -0.5
EXP_C = 5.0             # exp(s - M*SCALE + EXP_C): max weight e^5 = 148 < 240
                        # (TRN fp8e4 max normal is 240, unlike OCP e4m3fn's 448)
N_PX = 16384.0          # pixels per channel of the upsampled image
N_TOT = 128 * 16384.0   # elements per batch for layer stats

_CORES = list(range(8))
_REPLICA_GROUPS = [[0, 1, 2, 3], [4, 5, 6, 7]]


# ----------------------------------------------------------------------------
# Host-side helpers
# ----------------------------------------------------------------------------

def pos_encoding_pe(c, L, dtype=np.float32):
    half = c // 2
    pos = np.arange(L, dtype=dtype)
    depths = np.arange(half, dtype=dtype) / half
    rates = 1.0 / (10000.0 ** depths)
    ang = pos[:, None] * rates[None, :]
    pe = np.concatenate([np.sin(ang), np.cos(ang)], axis=-1)  # [L, c]
    return pe.T.astype(dtype)  # [c, L]


def reflect_pad(x):
    return np.pad(x, ((0, 0), (1, 1), (1, 1)), mode="reflect")


def edge_pad(x):
    return np.pad(x, ((0, 0), (1, 1), (1, 1)), mode="edge")


_KSET = {(0, 0): [0], (0, 1): [1, 2], (1, 0): [0, 1], (1, 1): [2]}


def collapse_w2(w):
    """w [co, ci, 3, 3] -> W2 [4 (p=2*pr+pc), 2 (dy), 2 (dx), ci, co]."""
    co, ci = w.shape[0], w.shape[1]
    W2 = np.zeros((4, 2, 2, ci, co), dtype=w.dtype)
    for pr in range(2):
        for pc in range(2):
            p = 2 * pr + pc
            for dy in range(2):
                for dx in range(2):
                    acc = np.zeros((co, ci), dtype=np.float64)
                    for ky in _KSET[(pr, dy)]:
                        for kx in _KSET[(pc, dx)]:
                            acc = acc + w[:, :, ky, kx].astype(np.float64)
                    W2[p, dy, dx] = acc.T.astype(w.dtype)
    return W2


def arrange_to_strips(x2d):
    """x [32, 128, 128] -> arranged [128, 4096] phase-major: partition
    32*(2*pr+pc)+c, free r*64+cc for upsampled pixel (2r+pr, 2cc+pc)."""
    t = x2d.reshape(32, 64, 2, 64, 2)          # c, r, pr, cc, pc
    t = t.transpose(2, 4, 0, 1, 3)              # pr, pc, c, r, cc
    return np.ascontiguousarray(t.reshape(128, 4096))


def unarrange_from_strips(arr):
    t = arr.reshape(2, 2, 32, 64, 64)           # pr, pc, c, r, cc
    t = t.transpose(2, 3, 0, 4, 1)              # c, r, pr, cc, pc
    return np.ascontiguousarray(t.reshape(32, 128, 128))


def make_w9(w):
    # W9[ey, ex][ci, 32*p+c] = W2[p, ey-pr, ex-pc][ci, c] (0 if invalid):
    # all four phases computed from one 9-tap pass over the edge-padded
    # original-resolution image, phase-major on output partitions.
    W2 = collapse_w2(w)                      # [4, 2, 2, ci, co32]
    ci = W2.shape[3]
    W9 = np.zeros((3, 3, ci, 128), dtype=np.float32)
    for p in range(4):
        pr, pc = p // 2, p % 2
        for dy in range(2):
            for dx in range(2):
                W9[pr + dy, pc + dx, :, 32 * p : 32 * p + 32] = W2[p, dy, dx]
    return W9


_PE_Y = None
_PE_S = None
_BATCH_CACHE = {}


def _batch_shared(inputs, b):
    key = (id(inputs), b)
    if key in _BATCH_CACHE:
        return _BATCH_CACHE[key]
    y = np.asarray(inputs["y"], dtype=np.float32)[b]
    s = np.asarray(inputs["s"], dtype=np.float32)[b]
    ypepad = np.ascontiguousarray(
        reflect_pad((y + _PE_Y).astype(np.float32)).reshape(2, 128, 66, 66)
    ).astype(BF16)
    yreppad = np.ascontiguousarray(edge_pad(y).reshape(2, 128, 66, 66)).astype(BF16)
    spepad = np.ascontiguousarray(
        reflect_pad((s + _PE_S).astype(np.float32))).astype(BF16)
    _BATCH_CACHE.clear()
    _BATCH_CACHE[key] = (ypepad, yreppad, spepad)
    return _BATCH_CACHE[key]


def prepare_core_inputs(inputs, core):
    global _PE_Y, _PE_S
    if _PE_Y is None:
        _PE_Y = pos_encoding_pe(256, 64 * 64).reshape(256, 64, 64)
        _PE_S = pos_encoding_pe(128, 128 * 128).reshape(128, 128, 128)
    b, h = core // 4, core % 4
    ch = slice(32 * h, 32 * h + 32)
    s = np.asarray(inputs["s"], dtype=np.float32)[b]

    ypepad, yreppad, spepad = _batch_shared(inputs, b)
    sgate = arrange_to_strips(np.ascontiguousarray(s[ch])).astype(BF16)

    w_blue_y = np.asarray(inputs["w_blue_y"], dtype=np.float32)[ch]
    w_blue_s = np.asarray(inputs["w_blue_s"], dtype=np.float32)[ch]
    w_green = np.asarray(inputs["w_green"], dtype=np.float32)[ch]
    w_purple = np.asarray(inputs["w_purple"], dtype=np.float32)[ch]

    # q conv: 18 steps (9 taps x 2 ci halves), out = 32 channels
    wq = np.zeros((18, 128, 32), dtype=np.float32)
    for t in range(9):
        ky, kx = t // 3, t % 3
        for kt in range(2):
            wq[t * 2 + kt] = w_blue_y[:, 128 * kt : 128 * kt + 128, ky, kx].T
    wv = np.zeros((9, 128, 32), dtype=np.float32)
    for t in range(9):
        ky, kx = t // 3, t % 3
        wv[t] = w_blue_s[:, :, ky, kx].T

    W9g = make_w9(w_green)                       # [3, 3, 256, 128]
    wg = W9g.reshape(3, 3, 2, 128, 128).reshape(18, 128, 128).copy()
    W9p = make_w9(w_purple)                      # [3, 3, 128, 128]
    wp = W9p.reshape(9, 128, 128).copy()

    # K-correction weights: Wsum[c, ci] = sum of all 9 taps; identical for
    # every phase of the collapsed conv.  WsumT [ci=128, pch=128].
    Wsum = w_purple.sum(axis=(2, 3))             # [32, 128]
    WsumT = np.zeros((128, 128), dtype=np.float32)
    for p in range(4):
        WsumT[:, 32 * p : 32 * p + 32] = Wsum.T

    affg = np.stack(
        [np.asarray(inputs["rho_g"], dtype=np.float32).reshape(128)[ch],
         np.asarray(inputs["gamma_g"], dtype=np.float32).reshape(128)[ch],
         np.asarray(inputs["beta_g"], dtype=np.float32).reshape(128)[ch]],
        axis=1)
    affp = np.stack(
        [np.asarray(inputs["rho_p"], dtype=np.float32).reshape(128)[ch],
         np.asarray(inputs["gamma_p"], dtype=np.float32).reshape(128)[ch],
         np.asarray(inputs["beta_p"], dtype=np.float32).reshape(128)[ch]],
        axis=1)

    sel = np.zeros((128, 32), dtype=np.float32)
    for p in range(128):
        sel[p, p % 32] = 1.0

    return {
        "ypepad": ypepad,
        "yreppad": yreppad,
        "spepad": spepad,
        "sgate": sgate,
        "wq": wq.astype(BF16),
        "wv": wv.astype(BF16),
        "wg": wg.astype(BF16),
        "wp": wp.astype(BF16),
        "wsumt": WsumT,
        "affg": np.ascontiguousarray(affg),
        "affp": np.ascontiguousarray(affp),
        "sel": sel,
    }


def assemble_output(per_core_z, per_core_upy):
    out = np.zeros((2, 256, 128, 128), dtype=np.float32)
    for core in range(8):
        b, h = core // 4, core % 4
        z = np.asarray(per_core_z[core]).astype(np.float32)
        u = np.asarray(per_core_upy[core]).astype(np.float32)
        out[b, 32 * h : 32 * h + 32] = unarrange_from_strips(z)
        out[b, 128 + 32 * h : 128 + 32 * h + 32] = unarrange_from_strips(u)
    return out


# ----------------------------------------------------------------------------
# Bass kernel
# ----------------------------------------------------------------------------

def build_bass(no_cc=False):
    import concourse.bass as bass
    import concourse.tile as tile
    from concourse import bacc, mybir

    f32 = mybir.dt.float32
    f32r = mybir.dt.float32r
    bf16 = mybir.dt.bfloat16
    fp8 = mybir.dt.float8e4
    AF = mybir.ActivationFunctionType
    ALU = mybir.AluOpType
    DR = mybir.MatmulPerfMode.DoubleRow

    nc = bacc.Bacc(num_devices=8)

    # ---- I/O ----
    ypepad_d = nc.declare_dram_parameter("ypepad", [2, 128, 66, 66], bf16, isOutput=False)
    yreppad_d = nc.declare_dram_parameter("yreppad", [2, 128, 66, 66], bf16, isOutput=False)
    spepad_d = nc.declare_dram_parameter("spepad", [128, 130, 130], bf16, isOutput=False)
    sgate_d = nc.declare_dram_parameter("sgate", [128, 4096], bf16, isOutput=False)
    wq_d = nc.declare_dram_parameter("wq", [18, 128, 32], bf16, isOutput=False)
    wv_d = nc.declare_dram_parameter("wv", [9, 128, 32], bf16, isOutput=False)
    wg_d = nc.declare_dram_parameter("wg", [18, 128, 128], bf16, isOutput=False)
    wp_d = nc.declare_dram_parameter("wp", [9, 128, 128], bf16, isOutput=False)
    wsumt_d = nc.declare_dram_parameter("wsumt", [128, 128], f32r, isOutput=False)
    affg_d = nc.declare_dram_parameter("affg", [32, 3], f32, isOutput=False)
    affp_d = nc.declare_dram_parameter("affp", [32, 3], f32, isOutput=False)
    sel_d = nc.declare_dram_parameter("sel", [128, 32], f32, isOutput=False)
    zout_d = nc.declare_dram_parameter("zout", [128, 4096], bf16, isOutput=True)
    upyout_d = nc.declare_dram_parameter("upyout", [128, 4096], bf16, isOutput=True)

    # ---- internal DRAM ----
    # gather #1: dev chunks for I-blocks 0..5; #2: I-blocks 6,7 + metadata.
    # meta (bf16 cols of cc1b): c bitcast-f32 at [p, 1024:1026]; green layer
    # sums bitcast-f32 at [0, 1026:1030] (sum, sumsq).
    cc1a_in = nc.dram_tensor("cc1a_in", [32, 3072], bf16)
    cc1a_out = nc.dram_tensor("cc1a_out", [128, 3072], bf16)
    cc1b_in = nc.dram_tensor("cc1b_in", [32, 1032], bf16)
    cc1b_out = nc.dram_tensor("cc1b_out", [128, 1032], bf16)
    cc2_in = nc.dram_tensor("cc2_in", [1, 2], f32)
    cc2_out = nc.dram_tensor("cc2_out", [1, 2], f32)
    rsumb = nc.dram_tensor("rsumb", [4096], f32)
    maxb = nc.dram_tensor("maxb", [1], f32)
    glb = nc.dram_tensor("glb", [2], f32)

    import contextlib

    with tile.TileContext(nc) as tc, contextlib.ExitStack() as ctx:
        pers = ctx.enter_context(tc.tile_pool(name="pers", bufs=1))
        small = ctx.enter_context(tc.tile_pool(name="small", bufs=1))

        def rsqrt_col(x, p, tag, eps=EPS):
            """[p, 1] tile -> rsqrt(x + eps), reciprocal + Sqrt ACT + one
            Newton step."""
            xe = small.tile([p, 1], f32, tag=tag + "xe", name=tag + "xe")
            nc.vector.tensor_scalar_add(xe, x, eps)
            r = small.tile([p, 1], f32, tag=tag + "r", name=tag + "r")
            nc.vector.reciprocal(out=r, in_=xe)
            y = small.tile([p, 1], f32, tag=tag + "y", name=tag + "y")
            nc.scalar.activation(out=y, in_=r, func=AF.Sqrt)
            t = small.tile([p, 1], f32, tag=tag + "nt", name=tag + "nt")
            nc.vector.tensor_mul(t, y, y)
            nc.vector.tensor_mul(t, t, xe)
            nc.vector.tensor_scalar(out=t, in0=t, scalar1=-0.5, scalar2=1.5,
                                    op0=ALU.mult, op1=ALU.add)
            nc.vector.tensor_mul(y, y, t)
            return y

        # ---------------- persistent weights/constants ----------------
        wq_sb = pers.tile([128, 18, 32], bf16, tag="wq")
        nc.sync.dma_start(out=wq_sb, in_=wq_d[:, :, :].rearrange("t p m -> p t m"))
        wv_sb = pers.tile([128, 9, 32], bf16, tag="wv")
        nc.sync.dma_start(out=wv_sb, in_=wv_d[:, :, :].rearrange("t p m -> p t m"))
        wg_sb = pers.tile([128, 18, 128], bf16, tag="wg")
        nc.sync.dma_start(out=wg_sb, in_=wg_d[:, :, :].rearrange("t p m -> p t m"))
        wp_sb = pers.tile([128, 9, 128], bf16, tag="wp")
        nc.sync.dma_start(out=wp_sb, in_=wp_d[:, :, :].rearrange("t p m -> p t m"))
        wsumt_sb = pers.tile([128, 128], f32r, tag="wsumt")
        nc.sync.dma_start(out=wsumt_sb, in_=wsumt_d[:, :])
        sel_sb = pers.tile([128, 32], f32, tag="sel")
        nc.sync.dma_start(out=sel_sb, in_=sel_d[:, :])
        ones_sb = pers.tile([128, 1], f32, tag="ones")
        nc.vector.memset(ones_sb, 1.0)
        ones32r = pers.tile([32, 1], f32r, tag="ones32")
        nc.vector.tensor_copy(ones32r, ones_sb[0:32, :])
        affg_sb = small.tile([32, 3], f32, tag="affg")
        nc.sync.dma_start(out=affg_sb, in_=affg_d[:, :])
        affp_sb = small.tile([32, 3], f32, tag="affp")
        nc.sync.dma_start(out=affp_sb, in_=affp_d[:, :])

        # persistent attention operands + stats
        q8 = pers.tile([16, 2, 4096], fp8, tag="q8")
        # dual-fp8 LDWEIGHTS requires a full 128-column weight tile, so the
        # inner dim is padded with zeros; col 32 = ones (rowsum).
        vT8 = pers.tile([128, 32, 128], fp8, tag="vT8")
        ebias = small.tile([128, 1], f32, tag="ebias")
        qstats = small.tile([32, 8, 6], f32, tag="qstats")
        vstats = small.tile([32, 8, 6], f32, tag="vstats")
        gstats = small.tile([128, 8, 6], f32, tag="gstats")

        qv_pool = ctx.enter_context(tc.tile_pool(name="qv", bufs=1))
        qraw = qv_pool.tile([32, 4096], f32, tag="qraw", name="qraw")
        vraw = qv_pool.tile([32, 4096], f32, tag="vraw", name="vraw")

        # =========== q conv (blue_y, bf16, out 32ch) ===========
        with tc.tile_pool(name="qsec", bufs=1) as qsec, \
             tc.tile_pool(name="qps", bufs=2, space="PSUM") as qps_pool:
            ype = [qsec.tile([128, 66, 66], bf16, tag=f"ype{kt}", name=f"ype{kt}")
                   for kt in range(2)]
            for kt in range(2):
                for rb in range(3):
                    r0, r1 = 22 * rb, 22 * rb + 22
                    nc.sync.dma_start(out=ype[kt][:, r0:r1, :], in_=ypepad_d[kt][:, r0:r1, :])
            for chunk in range(8):
                qps = qps_pool.tile([32, 512], f32, tag="qps", name=f"qps{chunk}")
                r0 = 8 * chunk
                idx = 0
                for t in range(9):
                    ky, kx = t // 3, t % 3
                    for kt in range(2):
                        nc.tensor.matmul(
                            qps[:, :],
                            wq_sb[:, t * 2 + kt, :],
                            ype[kt][:, r0 + ky : r0 + ky + 8, kx : kx + 64],
                            start=(idx == 0), stop=(idx == 17),
                        )
                        idx += 1
                col = 512 * chunk
                nc.vector.tensor_copy(qraw[:, col : col + 512], qps[:, :])
                nc.vector.bn_stats(out=qstats[:, chunk, :],
                                   in_=qraw[:, col : col + 512])

        # =========== v conv (blue_s, stride 2, bf16, out 32ch) ===========
        with tc.tile_pool(name="vsec", bufs=1) as vsec, \
             tc.tile_pool(name="vps", bufs=2, space="PSUM") as vps_pool:
            spe = vsec.tile([128, 130, 130], bf16, tag="spe")
            for rb in range(5):
                r0, r1 = 26 * rb, 26 * rb + 26
                nc.sync.dma_start(out=spe[:, r0:r1, :], in_=spepad_d[:, r0:r1, :])
            for chunk in range(8):
                vps = vps_pool.tile([32, 512], f32, tag="vps", name=f"vps{chunk}")
                r0 = 8 * chunk
                for t in range(9):
                    ky, kx = t // 3, t % 3
                    nc.tensor.matmul(
                        vps[:, :],
                        wv_sb[:, t, :],
                        spe[:, 2 * r0 + ky : 2 * r0 + ky + 16 : 2, kx : kx + 128 : 2],
                        start=(t == 0), stop=(t == 8),
                    )
                col = 512 * chunk
                nc.vector.tensor_copy(vraw[:, col : col + 512], vps[:, :])
                nc.vector.bn_stats(out=vstats[:, chunk, :],
                                   in_=vraw[:, col : col + 512])

        # =========== norms + silus + fp8 packs + exp bias ===========
        with tc.tile_pool(name="prep", bufs=1) as prep:
            qmv = small.tile([32, 2], f32, tag="qmv")
            nc.vector.bn_aggr(out=qmv, in_=qstats)
            vmv = small.tile([32, 2], f32, tag="vmv")
            nc.vector.bn_aggr(out=vmv, in_=vstats)
            qinv = rsqrt_col(qmv[:, 1:2], 32, "qinv")
            vinv = rsqrt_col(vmv[:, 1:2], 32, "vinv")
            qbias = small.tile([32, 1], f32, tag="qbias")
            nc.vector.tensor_scalar(out=qbias, in0=qmv[:, 0:1], scalar1=qinv,
                                    scalar2=-1.0, op0=ALU.mult, op1=ALU.mult)
            vbias = small.tile([32, 1], f32, tag="vbias")
            nc.vector.tensor_scalar(out=vbias, in0=vmv[:, 0:1], scalar1=vinv,
                                    scalar2=-1.0, op0=ALU.mult, op1=ALU.mult)

            q8unp = prep.tile([32, 4096], fp8, tag="q8unp")
            nc.scalar.activation(out=q8unp, in_=qraw, func=AF.Silu,
                                 bias=qbias, scale=qinv)
            v2d = prep.tile([32, 4096], f32, tag="v2d")
            nc.scalar.activation(out=v2d, in_=vraw, func=AF.Silu,
                                 bias=vbias, scale=vinv)

            # pack q8[p, r, i] = q8unp[2p+r, i]
            nc.sync.dma_start(out=q8[:, :, :], in_=q8unp[:, :])

            # M = max_i sum_d q~[d,i]^2 ; ebias = EXP_C - M*SCALE
            qsq = prep.tile([32, 4096], f32r, tag="qsq")
            nc.scalar.activation(out=qsq, in_=q8unp, func=AF.Square)
            cmax = small.tile([1, 8, 8], f32, tag="cmax")
            with tc.tile_pool(name="mps", bufs=2, space="PSUM") as mps_pool:
                for chunk in range(8):
                    mps = mps_pool.tile([1, 512], f32, tag="mps", name=f"mps{chunk}")
                    nc.tensor.matmul(
                        mps[:, :], ones32r,
                        qsq[:, 512 * chunk : 512 * chunk + 512],
                        start=True, stop=True)
                    nc.vector.max(out=cmax[:, chunk, :], in_=mps[:, :])
            m8 = small.tile([1, 8], f32, tag="m8")
            nc.vector.max(out=m8, in_=cmax.rearrange("p a b -> p (a b)"))
            nc.sync.dma_start(out=bass.AP(tensor=maxb, offset=0, ap=[[1, 1]]),
                              in_=m8[0:1, 0:1])
            nc.sync.dma_start(out=ebias,
                              in_=bass.AP(tensor=maxb, offset=0, ap=[[0, 128], [1, 1]]))
            nc.vector.tensor_scalar(out=ebias, in0=ebias, scalar1=-SCALE,
                                    scalar2=EXP_C, op0=ALU.mult, op1=ALU.add)

            # vT staging: vTf[32w+i, jb, c] = v2d[c, 128*jb+32w+i]; + ones col
            vt32 = prep.tile([32, 4096], f32, tag="vt32")
            nc.vector.transpose(out=vt32, in_=v2d)
            vTf = prep.tile([128, 32, 128], f32, tag="vTf")
            nc.vector.memset(vTf[:, :, 33:128], 0.0)
            nc.vector.memset(vTf[:, :, 32:33], 1.0)
            vt32_v = vt32.rearrange("p (m c) -> p m c", c=32)   # m = 4*jb + w
            for w in range(4):
                nc.gpsimd.dma_start(
                    out=vTf[32 * w : 32 * w + 32, :, 0:32],
                    in_=vt32_v[:, w::4, :])
            nc.vector.tensor_copy(vT8[:, :, :], vTf[:, :, :])

        # =========== attention loop + interleaved green conv ===========
        greenraw = ctx.enter_context(tc.tile_pool(name="gpool", bufs=1)).tile(
            [128, 4096], f32, tag="greenraw", name="greenraw")
        devbuf = ctx.enter_context(tc.tile_pool(name="devpool", bufs=1)).tile(
            [32, 1032], bf16, tag="devbuf", name="devbuf")
        c_q = small.tile([32, 1], f32, tag="c_q")

        with tc.tile_pool(name="yrep", bufs=1) as yrep_pool, \
             tc.tile_pool(name="aexpp", bufs=6) as aexp_pool, \
             tc.tile_pool(name="mhch", bufs=2) as mhch_pool, \
             tc.tile_pool(name="rsp", bufs=2) as rsp_pool, \
             tc.tile_pool(name="gps", bufs=2, space="PSUM") as gps_pool, \
             tc.tile_pool(name="qkps", bufs=2, space="PSUM") as qkps, \
             tc.tile_pool(name="pvps", bufs=2, space="PSUM") as pvps:
            yrep = [yrep_pool.tile([128, 66, 66], bf16, tag=f"yrep{kt}", name=f"yrep{kt}")
                    for kt in range(2)]
            for kt in range(2):
                for rb in range(3):
                    r0, r1 = 22 * rb, 22 * rb + 22
                    nc.sync.dma_start(out=yrep[kt][:, r0:r1, :], in_=yreppad_d[kt][:, r0:r1, :])

            # green: 8 chunks x 18 taps of [128, 512]-out bf16 matmuls = 144
            green_tiles = {}

            def green_mm(idx):
                chunk, step = idx // 18, idx % 18
                r0 = 8 * chunk
                if step == 0:
                    green_tiles[chunk] = gps_pool.tile(
                        [128, 512], f32, tag="gpsum", name=f"g{chunk}")
                gtile = green_tiles[chunk]
                tap, kt = step // 2, step % 2
                ey, ex = tap // 3, tap % 3
                nc.tensor.matmul(
                    gtile[:, :],
                    wg_sb[:, tap * 2 + kt, :],
                    yrep[kt][:, r0 + ey : r0 + ey + 8, ex : ex + 64],
                    start=(step == 0), stop=(step == 17),
                )
                if step == 17:
                    col = 512 * chunk
                    nc.vector.tensor_copy(greenraw[:, col : col + 512], gtile[:, :])
                    nc.vector.bn_stats(out=gstats[:, chunk, :],
                                       in_=greenraw[:, col : col + 512])
                    del green_tiles[chunk]

            def epilogue_I(I, pvt):
                """rowsum reciprocal, divide, dev-quantize, stage for gather."""
                mhc = mhch_pool.tile([33, 512], f32, tag="mhc", name=f"mhc{I}")
                nc.vector.tensor_copy(mhc, pvt[0:33, :])
                rs1 = rsp_pool.tile([1, 512], f32, tag="rs1", name=f"rs1_{I}")
                nc.vector.reciprocal(out=rs1, in_=mhc[32:33, :])
                nc.sync.dma_start(
                    out=bass.AP(tensor=rsumb, offset=512 * I, ap=[[1, 512]]), in_=rs1)
                rs32 = rsp_pool.tile([32, 512], f32, tag="rs32", name=f"rs32_{I}")
                nc.sync.dma_start(
                    out=rs32,
                    in_=bass.AP(tensor=rsumb, offset=512 * I, ap=[[0, 32], [1, 512]]))
                nc.vector.tensor_mul(mhc[0:32, :], mhc[0:32, :], rs32)
                if I == 0:
                    st = small.tile([32, 1, 6], f32, tag="cstats")
                    nc.vector.bn_stats(out=st, in_=mhc[0:32, :])
                    cmv = small.tile([32, 2], f32, tag="cmv")
                    nc.vector.bn_aggr(out=cmv, in_=st)
                    nc.vector.tensor_copy(c_q, cmv[:, 0:1])
                    nc.vector.tensor_copy(devbuf[:, 1024:1026],
                                          c_q[:, 0:1].bitcast(bf16))
                if I < 6:
                    dev = rsp_pool.tile([32, 512], bf16, tag="dev", name=f"dev{I}")
                    nc.vector.tensor_scalar_sub(dev, mhc[0:32, :], c_q)
                    nc.sync.dma_start(out=cc1a_in[:, 512 * I : 512 * I + 512], in_=dev)
                else:
                    nc.vector.tensor_scalar_sub(
                        devbuf[:, 512 * (I - 6) : 512 * (I - 6) + 512],
                        mhc[0:32, :], c_q)

            gidx = 0
            for I in range(8):
                pvt = pvps.tile([128, 512], f32, tag="pvt", name=f"pvt{I}")
                for g in range(16):
                    qk = qkps.tile([128, 1024], f32, tag="qk", name=f"qk{I}_{g}")
                    for t in range(2):
                        jb = 2 * g + t
                        nc.tensor.matmul(
                            qk[:, 512 * t : 512 * t + 512],
                            q8[:, :, 128 * jb : 128 * jb + 128],
                            q8[:, :, 512 * I : 512 * I + 512],
                            start=True, stop=True,
                            perf_mode=DR,
                        )
                    aexp = aexp_pool.tile([128, 1024], fp8, tag="aexp", name=f"ae{I}_{g}")
                    nc.scalar.activation(out=aexp, in_=qk, func=AF.Exp,
                                         bias=ebias, scale=SCALE)
                    nc.tensor.matmul(
                        pvt[:, :],
                        vT8[:, 2 * g : 2 * g + 2, :],
                        aexp.rearrange("p (r i) -> p r i", r=2),
                        start=(g == 0), stop=(g == 15),
                        perf_mode=DR,
                        skip_group_check=True,
                    )
                    # interleave green: 144 MMs over 128 iters
                    it = 16 * I + g
                    n_target = ((it + 1) * 144) // 128
                    while gidx < n_target:
                        green_mm(gidx)
                        gidx += 1
                epilogue_I(I, pvt)
                if I == 5:
                    if no_cc:
                        for gg in range(4):
                            nc.sync.dma_start(
                                out=cc1a_out[32 * gg : 32 * gg + 32, :],
                                in_=cc1a_in[:, :])
                    else:
                        nc.gpsimd.collective_compute(
                            "AllGather", mybir.AluOpType.bypass,
                            replica_groups=_REPLICA_GROUPS,
                            ins=[cc1a_in[:, :]],
                            outs=[cc1a_out[:, :]],
                        )

            # green layer sums ride gather #2
            gmv = small.tile([128, 2], f32, tag="gmv")
            nc.vector.bn_aggr(out=gmv, in_=gstats)

            def part_sums(mv, p, tag):
                s2 = small.tile([p, 2], f32, tag=tag, name=tag)
                nc.vector.tensor_scalar_mul(s2[:, 0:1], mv[:, 0:1], 4096.0)
                t = small.tile([p, 1], f32, tag=tag + "t", name=tag + "t")
                nc.vector.tensor_mul(t, mv[:, 0:1], mv[:, 0:1])
                nc.vector.tensor_add(t, t, mv[:, 1:2])
                nc.vector.tensor_scalar_mul(s2[:, 1:2], t, 4096.0)
                return s2

            gsums2 = part_sums(gmv, 128, "gsums2")

            lps = pvps.tile([128, 512], f32, tag="pvt", name="lps")
            nc.tensor.matmul(lps[0:1, 0:2], ones_sb, gsums2, start=True, stop=True)
            glsum = small.tile([1, 2], f32, tag="glsum")
            nc.vector.tensor_copy(glsum, lps[0:1, 0:2])
            nc.vector.tensor_copy(devbuf[0:1, 1026:1030], glsum.bitcast(bf16))
            chps = pvps.tile([128, 512], f32, tag="pvt", name="chps")
            nc.tensor.matmul(chps[0:32, 0:2], sel_sb, gsums2, start=True, stop=True)
            gch = small.tile([32, 2], f32, tag="gch")
            nc.vector.tensor_copy(gch, chps[0:32, 0:2])

            nc.sync.dma_start(out=cc1b_in[:, :], in_=devbuf)
            if no_cc:
                for gg in range(4):
                    nc.sync.dma_start(
                        out=cc1b_out[32 * gg : 32 * gg + 32, :], in_=cc1b_in[:, :])
            else:
                nc.gpsimd.collective_compute(
                    "AllGather", mybir.AluOpType.bypass,
                    replica_groups=_REPLICA_GROUPS,
                    ins=[cc1b_in[:, :]],
                    outs=[cc1b_out[:, :]],
                )

        # =========== purple conv + tails ===========
        with tc.tile_pool(name="tailp", bufs=1) as tailp, \
             tc.tile_pool(name="gps2", bufs=2, space="PSUM") as gps2, \
             tc.tile_pool(name="tailps", bufs=2, space="PSUM") as tailps:
            # mhapad split: A rows 0..42, B rows 32..65 (of the 66-row padded
            # image); purple chunks 0..3 read A only (depends on gather #1),
            # chunks 4..7 read B (gather #2).
            mhapadA = tailp.tile([128, 43, 66], bf16, tag="mhapadA")
            mhapadB = tailp.tile([128, 34, 66], bf16, tag="mhapadB")
            cc1a_v = cc1a_out.rearrange("p (r c) -> p r c", c=64)   # data rows 0..47
            cc1b_v = cc1b_out[:, 0:1024].rearrange("p (r c) -> p r c", c=64)  # 48..63
            # A: pad row (=data row 0), data rows 0..41
            nc.sync.dma_start(out=mhapadA[:, 0:1, 1:65], in_=cc1a_v[:, 0:1, :])
            nc.sync.dma_start(out=mhapadA[:, 1:43, 1:65], in_=cc1a_v[:, 0:42, :])
            nc.sync.dma_start(out=mhapadA[:, :, 0:1], in_=mhapadA[:, :, 1:2])
            nc.sync.dma_start(out=mhapadA[:, :, 65:66], in_=mhapadA[:, :, 64:65])
            # B: data rows 31..47 (from #1), 48..63 (from #2), pad row (=63)
            nc.sync.dma_start(out=mhapadB[:, 0:17, 1:65], in_=cc1a_v[:, 31:48, :])
            nc.sync.dma_start(out=mhapadB[:, 17:33, 1:65], in_=cc1b_v)
            nc.sync.dma_start(out=mhapadB[:, 33:34, 1:65], in_=cc1b_v[:, 15:16, :])
            nc.sync.dma_start(out=mhapadB[:, :, 0:1], in_=mhapadB[:, :, 1:2])
            nc.sync.dma_start(out=mhapadB[:, :, 65:66], in_=mhapadB[:, :, 64:65])

            # K = WsumT^T @ c_full (c_full from bitcast meta)
            cmeta = tailp.tile([128, 2], bf16, tag="cmeta")
            nc.sync.dma_start(out=cmeta, in_=cc1b_out[:, 1024:1026])
            cfull = tailp.tile([128, 2], f32r, tag="cfull")
            nc.vector.tensor_copy(cfull[:, 0:1], cmeta.bitcast(f32))
            nc.vector.tensor_copy(cfull[:, 1:2], cmeta.bitcast(f32))
            K128 = small.tile([128, 1], f32, tag="K128")
            kps = tailps.tile([128, 512], f32, tag="tps", name="kps")
            nc.tensor.matmul(kps[:, 0:2], wsumt_sb, cfull,
                             start=True, stop=True)
            nc.vector.tensor_copy(K128, kps[:, 0:1])

            # green layer sums from the 4 cores' meta
            glmeta = tailp.tile([1, 4, 4], bf16, tag="glmeta")
            nc.sync.dma_start(
                out=glmeta,
                in_=bass.AP(tensor=cc1b_out, offset=1026,
                            ap=[[0, 1], [32 * 1032, 4], [1, 4]]))
            glf = small.tile([1, 4, 2], f32, tag="glf")
            nc.vector.tensor_copy(glf, glmeta.bitcast(f32))
            gl1 = small.tile([1, 2], f32, tag="gl1")
            nc.vector.tensor_add(gl1, glf[:, 0, :], glf[:, 1, :])
            gl2 = small.tile([1, 2], f32, tag="gl2")
            nc.vector.tensor_add(gl2, glf[:, 2, :], glf[:, 3, :])
            nc.vector.tensor_add(gl1, gl1, gl2)
            nc.sync.dma_start(out=bass.AP(tensor=glb, offset=0, ap=[[1, 2]]), in_=gl1)
            glbc = small.tile([32, 2], f32, tag="glbc")
            nc.sync.dma_start(out=glbc,
                              in_=bass.AP(tensor=glb, offset=0, ap=[[0, 32], [1, 2]]))

            purpleraw = tailp.tile([128, 4096], f32, tag="purpleraw")
            pstats = small.tile([128, 8, 6], f32, tag="pstats")
            for chunk in range(8):
                ptile = gps2.tile([128, 512], f32, tag="gpsum2", name=f"pt{chunk}")
                r0 = 8 * chunk
                src = mhapadA if chunk < 4 else mhapadB
                roff = 0 if chunk < 4 else 32
                for tap in range(9):
                    ey, ex = tap // 3, tap % 3
                    nc.tensor.matmul(
                        ptile[:, :],
                        wp_sb[:, tap, :],
                        src[:, r0 - roff + ey : r0 - roff + ey + 8, ex : ex + 64],
                        start=(tap == 0), stop=(tap == 8),
                    )
                col = 512 * chunk
                nc.vector.tensor_scalar_add(purpleraw[:, col : col + 512],
                                            ptile[:, :], K128)
                nc.vector.bn_stats(out=pstats[:, chunk, :],
                                   in_=purpleraw[:, col : col + 512])
            pmv = small.tile([128, 2], f32, tag="pmv")
            nc.vector.bn_aggr(out=pmv, in_=pstats)
            psums2 = part_sums(pmv, 128, "psums2")

            chps2 = tailps.tile([128, 512], f32, tag="tps", name="chps2")
            nc.tensor.matmul(chps2[0:32, 0:2], sel_sb, psums2, start=True, stop=True)
            pch = small.tile([32, 2], f32, tag="pch")
            nc.vector.tensor_copy(pch, chps2[0:32, 0:2])

            lps2 = tailps.tile([128, 512], f32, tag="tps", name="lps2")
            nc.tensor.matmul(lps2[0:1, 0:2], ones_sb, psums2, start=True, stop=True)
            plsum = small.tile([1, 2], f32, tag="plsum")
            nc.vector.tensor_copy(plsum, lps2[0:1, 0:2])
            nc.sync.dma_start(out=cc2_in[:, :], in_=plsum)
            if no_cc:
                nc.sync.dma_start(out=cc2_out[:, :], in_=cc2_in[:, :])
            else:
                nc.gpsimd.collective_compute(
                    "AllReduce", mybir.AluOpType.add,
                    replica_groups=_REPLICA_GROUPS,
                    ins=[cc2_in[:, :]],
                    outs=[cc2_out[:, :]],
                )
            plng = small.tile([32, 2], f32, tag="plng")
            nc.sync.dma_start(out=plng,
                              in_=bass.AP(tensor=cc2_out, offset=0, ap=[[0, 32], [1, 2]]))

            # ---- ILN affines ----
            def iln_affine(ch_sums, S_col, aff_sb, tag):
                n, n1 = N_PX, N_PX - 1.0
                nt, nt1 = N_TOT, N_TOT - 1.0
                in_m = small.tile([32, 1], f32, tag=tag + "im", name=tag + "im")
                nc.vector.tensor_scalar_mul(in_m, ch_sums[:, 0:1], 1.0 / n)
                t1 = small.tile([32, 1], f32, tag=tag + "t1", name=tag + "t1")
                nc.vector.tensor_mul(t1, ch_sums[:, 0:1], ch_sums[:, 0:1])
                nc.vector.tensor_scalar_mul(t1, t1, 1.0 / n)
                nc.vector.tensor_sub(t1, ch_sums[:, 1:2], t1)
                in_v = small.tile([32, 1], f32, tag=tag + "iv", name=tag + "iv")
                nc.vector.tensor_scalar_mul(in_v, t1, 1.0 / n1)
                inv_in = rsqrt_col(in_v, 32, tag + "ii")

                ln_m = small.tile([32, 1], f32, tag=tag + "lm", name=tag + "lm")
                nc.vector.tensor_scalar_mul(ln_m, S_col[:, 0:1], 1.0 / nt)
                l1 = small.tile([32, 1], f32, tag=tag + "l1", name=tag + "l1")
                nc.vector.tensor_mul(l1, S_col[:, 0:1], S_col[:, 0:1])
                nc.vector.tensor_scalar_mul(l1, l1, 1.0 / nt)
                nc.vector.tensor_sub(l1, S_col[:, 1:2], l1)
                ln_v = small.tile([32, 1], f32, tag=tag + "lv", name=tag + "lv")
                nc.vector.tensor_scalar_mul(ln_v, l1, 1.0 / nt1)
                inv_ln = rsqrt_col(ln_v, 32, tag + "il")

                rho = aff_sb[:, 0:1]
                t3 = small.tile([32, 1], f32, tag=tag + "t3", name=tag + "t3")
                nc.vector.tensor_mul(t3, rho, inv_in)
                t6 = small.tile([32, 1], f32, tag=tag + "t6", name=tag + "t6")
                nc.vector.tensor_mul(t6, rho, inv_ln)
                nc.vector.tensor_sub(t6, inv_ln, t6)
                A = small.tile([32, 1], f32, tag=tag + "A", name=tag + "A")
                nc.vector.tensor_add(A, t3, t6)
                u1 = small.tile([32, 1], f32, tag=tag + "u1", name=tag + "u1")
                nc.vector.tensor_mul(u1, in_m, t3)
                u2 = small.tile([32, 1], f32, tag=tag + "u2", name=tag + "u2")
                nc.vector.tensor_mul(u2, ln_m, t6)
                nc.vector.tensor_add(u1, u1, u2)
                B = small.tile([32, 1], f32, tag=tag + "B", name=tag + "B")
                nc.vector.tensor_scalar_mul(B, u1, -1.0)
                sb = small.tile([32, 2], f32, tag=tag + "sb", name=tag + "sb")
                nc.vector.tensor_mul(sb[:, 0:1], A, aff_sb[:, 1:2])
                nc.vector.tensor_mul(sb[:, 1:2], B, aff_sb[:, 1:2])
                nc.vector.tensor_add(sb[:, 1:2], sb[:, 1:2], aff_sb[:, 2:3])
                return sb

            gsb = iln_affine(gch, glbc, affg_sb, "ga")
            psb = iln_affine(pch, plng, affp_sb, "pa")

            gsb128 = small.tile([128, 2], f32, tag="gsb128")
            psb128 = small.tile([128, 2], f32, tag="psb128")
            nc.sync.dma_start(out=gsb128[0:32, :], in_=gsb)
            nc.sync.dma_start(out=psb128[0:32, :], in_=psb)
            for o in (32, 64, 96):
                nc.sync.dma_start(out=gsb128[o : o + 32, :], in_=gsb128[0:32, :])
                nc.sync.dma_start(out=psb128[o : o + 32, :], in_=psb128[0:32, :])

            # ---- finalize outputs ----
            sgate_sb = tailp.tile([128, 4096], bf16, tag="sgate")
            nc.sync.dma_start(out=sgate_sb, in_=sgate_d[:, :])

            upy_sb = tailp.tile([128, 4096], bf16, tag="upy")
            nc.scalar.activation(out=upy_sb, in_=greenraw, func=AF.Silu,
                                 bias=gsb128[:, 1:2], scale=gsb128[:, 0:1])
            nc.sync.dma_start(out=upyout_d[:, :], in_=upy_sb)

            zpre = tailp.tile([128, 4096], bf16, tag="zpre")
            nc.scalar.activation(out=zpre, in_=purpleraw, func=AF.Sigmoid,
                                 bias=psb128[:, 1:2], scale=psb128[:, 0:1])
            nc.vector.tensor_mul(zpre, zpre, sgate_sb)
            nc.sync.dma_start(out=zout_d[:, :], in_=zpre)

    nc.compile()
    return nc


_NC_CACHE = None
RUN_KWARGS = {}      # test harness may set e.g. {"trace": True}
LAST_RESULTS = None  # BassKernelResults of the most recent run


def kernel(**inputs) -> np.ndarray:
    global _NC_CACHE, LAST_RESULTS
    from concourse.bass_utils import run_bass_kernel_spmd

    if _NC_CACHE is None:
        _NC_CACHE = build_bass()
    nc = _NC_CACHE

    in_maps = []
    for core in _CORES:
        ci = prepare_core_inputs(inputs, core)
        in_maps.append(ci)

    res = run_bass_kernel_spmd(nc, in_maps, _CORES, **RUN_KWARGS)
    LAST_RESULTS = res
    zs = [res.results[c]["zout"] for c in _CORES]
    upys = [res.results[c]["upyout"] for c in _CORES]
    return assemble_output(zs, upys)


if __name__ == "__main__":
    nc = build_bass()
    print("bass build OK")
